# revision 1
# baseline (speedup 1.0000x reference)
"""Multi-head attention (B=4, S=2048, D=1024, H=16, HD=64) on 8 TRN2 NeuronCores.

Sharding: core c handles batch b = c//2 and head-group hg = c%2 (8 heads each).
Attention is embarrassingly parallel over (b, head-group); the QKV projection is
column-sharded per core (tensor parallel on heads).

Per-core dataflow (everything in "transposed" layout to avoid on-chip transposes):
  - Host passes X^T [D, S] (f32), W slices in natural [D, cols] layout.
  - Projection:  Q^T/K^T [1024, S] = W_qk^T @ X accumulated in SBUF tiles (sbt),
                 which the attention stage reads directly as Q^T/K^T;
                 V [S, 512] = X @ W_v, kept in SBUF augmented with a ones-column
                 per head (V').
  - Per head:    S^T[k,q] = K^T.T @ Q^T  (PSUM, fp32)
                 st = exp(S^T / 8)       (ScalarE, fused scale; mask is all-ones and
                                          softmax is shift-invariant => no max pass)
                 out^T[d,q], sums[q] = V'^T @ st  (ones-row of V' yields softmax sums)
                 out^T[d,q] /= sums[q]   (DVE reciprocal + gpsimd partition broadcast)
  - Host transposes per-core out^T [512, S] back and concatenates.

The Q/K projection is sliced into ~1.7us "quanta" (4 contraction chunks of one
m-tile n-half) that are woven thinly into the attention chunk loop on a static
schedule, so the in-order PE stream fills the bubbles left by the ScalarE-paced
softmax. m-tile pair p is produced during heads 2p-2/2p-1, finishing before the
pair of heads that reads it. The V projection and m-tiles 0/4 run as a short
prefix that trickles in behind the X^T DMA stream. Within each head, the AV
matmuls are software-pipelined one chunk behind QK/exp so the PE never idles
waiting for ScalarE before issuing the next chunk's QK.

All matmuls run in float32r (fp32 data, ~1e-3 matmul rel err, bf16-class speed).
Projection partial sums accumulate in SBUF via DVE (float32r rounding per step).
b_qkv is applied (it is zeros in practice); mask is all-True per the problem spec
and is ignored.
"""

import numpy as np

import concourse.bass as bass
import concourse.mybir as mybir
import concourse.tile as tile
from concourse import bacc
from concourse.bass_utils import run_bass_kernel_spmd

F32 = mybir.dt.float32
F32R = mybir.dt.float32r
AF = mybir.ActivationFunctionType
ALU = mybir.AluOpType

P = 128          # partitions
D = 1024         # model dim
S = 2048         # sequence
HD = 64          # head dim
NHC = 8          # heads per core
QKC = NHC * HD   # 512 columns per core for each of Q, K, V
KD = D // P      # 8 contraction chunks
MS = S // P      # 16 sequence chunks
NQ = S // 512    # 4 q-tiles of 512
SCALE = 1.0 / 8.0  # 1/sqrt(HD)

N_CORES = 8
B_FULL, H_FULL = 4, 16


def _build(iters=1):
    nc = bacc.Bacc(None, target_bir_lowering=False)

    xt = nc.dram_tensor("xt", [D, S], F32R, kind="ExternalInput")
    # wqk is host-permuted: row (m*128 + p), col (k*128 + j) holds
    # W_qk[k*128 + p, m*128 + j] — so one m-tile's weights are a contiguous
    # [128, 1024] block (4KB DMA lines instead of 512B strided reads)
    wqk = nc.dram_tensor("wqk", [D, 2 * QKC], F32R, kind="ExternalInput")
    wv = nc.dram_tensor("wv", [D, QKC], F32R, kind="ExternalInput")
    bqk = nc.dram_tensor("bqk", [2 * QKC], F32, kind="ExternalInput")
    bv = nc.dram_tensor("bv", [QKC], F32, kind="ExternalInput")
    outT = nc.dram_tensor("outT", [QKC, S], F32, kind="ExternalOutput")

    with tile.TileContext(nc) as tc:
        with (
            tc.tile_pool(name="persist", bufs=1) as pp,
            tc.tile_pool(name="sbtp", bufs=4) as sbtp,
            tc.tile_pool(name="stp", bufs=6) as stp,
            tc.tile_pool(name="psc", bufs=2, space="PSUM") as psc,
            tc.tile_pool(name="psav", bufs=4, space="PSUM") as psav,
        ):
            # bias staging: bqk_sb[p, m] = bqk[m*128 + p]; bv broadcast across partitions
            bqk_sb = pp.tile([P, KD], F32, tag="bqk", name="bqk_sb")
            nc.sync.dma_start(out=bqk_sb[:], in_=bqk[:].rearrange("(m p) -> p m", p=P))
            bv_bc = pp.tile([P, QKC], F32, tag="bvb", name="bv_bc")
            nc.sync.dma_start(out=bv_bc[0:1, :], in_=bv[:].rearrange("(o n) -> o n", o=1))
            nc.gpsimd.partition_broadcast(bv_bc[:], bv_bc[0:1, :])
            ones8 = pp.tile([P, NHC], F32, tag="ones8", name="ones8")
            nc.vector.memset(ones8[:], 1.0)

            for it in range(iters):
                # V' tiles: [128 seq, 8 heads, 64+1] with ones in the last column
                v_sb = [
                    pp.tile([P, NHC, HD + 1], F32R, tag=f"v{k}", name=f"v{it}_{k}")
                    for k in range(MS)
                ]

                with tc.tile_pool(name=f"proj{it}", bufs=1) as pj:
                    w_tiles = {}

                    def load_wm(m, it=it):
                        w_tiles[m] = pj.tile([P, KD, P], F32R, tag="wm", bufs=3,
                                             name=f"wm{it}_{m}")
                        nc.sync.dma_start(
                            out=w_tiles[m][:],
                            in_=wqk[m * P:(m + 1) * P, :].rearrange("p (k j) -> p k j", k=KD))

                    load_wm(0)
                    load_wm(4)
                    xt_sb = [pj.tile([P, S], F32R, tag=f"xt{k}", name=f"xt{it}_{k}")
                             for k in range(KD)]
                    wv_sb = [pj.tile([P, QKC], F32R, tag=f"wv{k}", name=f"wv{it}_{k}")
                             for k in range(KD)]
                    # xt gates every prefix quantum — load it all first; wv is
                    # first needed ~13us in (V k-half 0) and fits behind it
                    for k in range(KD):
                        nc.sync.dma_start(out=xt_sb[k][:], in_=xt[k * P:(k + 1) * P, :])
                    for k in range(KD):
                        nc.sync.dma_start(out=wv_sb[k][:], in_=wv[k * P:(k + 1) * P, :])

                    sbt_tiles = {}

                    def qk_quantum(m, nh, kk, it=it, k0=None, nk=4, first=None):
                        """Accumulate nk k-chunks (from k0) of m-tile m, n-half nh
                        into the sbt SBUF accumulator (PE -> PSUM -> DVE add)."""
                        if m not in w_tiles:
                            load_wm(m)
                        if m not in sbt_tiles:
                            sbt_tiles[m] = sbtp.tile([P, S], F32R, tag="sbt",
                                                     name=f"sbt{it}_{m}")
                        w_m, sbt = w_tiles[m], sbt_tiles[m]
                        if k0 is None:
                            k0 = 4 * kk
                        if first is None:
                            first = (kk == 0)
                        ps = psc.tile([P, 1024], F32, tag="sc", name=f"pq{it}_{m}_{nh}_{k0}")
                        for j, k in enumerate(range(k0, k0 + nk)):
                            nc.tensor.matmul(
                                ps[:, 0:512], w_m[:, k, :],
                                xt_sb[k][:, nh * 1024: nh * 1024 + 512],
                                start=(j == 0), stop=(j == nk - 1))
                            nc.tensor.matmul(
                                ps[:, 512:1024], w_m[:, k, :],
                                xt_sb[k][:, nh * 1024 + 512:(nh + 1) * 1024],
                                start=(j == 0), stop=(j == nk - 1))
                        dst = sbt[:, nh * 1024:(nh + 1) * 1024]
                        if first:
                            nc.vector.tensor_scalar_add(dst, ps[:], bqk_sb[:, m:m + 1])
                        else:
                            nc.vector.tensor_tensor(out=dst, in0=ps[:], in1=dst, op=ALU.add)

                    def qk_quantum_fd(m, quarter, it=it):
                        """Full-depth steady quantum: one 512-wide quarter of
                        m-tile m, all 8 contraction chunks in one PSUM group,
                        a single DVE add — short slot hold, minimal DVE."""
                        if m not in w_tiles:
                            load_wm(m)
                        if m not in sbt_tiles:
                            sbt_tiles[m] = sbtp.tile([P, S], F32R, tag="sbt",
                                                     name=f"sbt{it}_{m}")
                        w_m, sbt = w_tiles[m], sbt_tiles[m]
                        ps = psc.tile([P, 512], F32, tag="sc", name=f"pq{it}_{m}_{quarter}")
                        for k in range(KD):
                            nc.tensor.matmul(
                                ps[:], w_m[:, k, :],
                                xt_sb[k][:, quarter * 512:(quarter + 1) * 512],
                                start=(k == 0), stop=(k == KD - 1))
                        nc.vector.tensor_scalar_add(
                            sbt[:, quarter * 512:(quarter + 1) * 512], ps[:],
                            bqk_sb[:, m:m + 1])

                    def prefix_m0_m4(it=it):
                        """m-tiles 0 and 4 together, k-major across 6 concurrent
                        full-depth PSUM groups (4 av-slot quarters for m0, 2
                        sc-slot halves for m4) — DMA-paced trickle, 6 DVE adds."""
                        for m in (0, 4):
                            sbt_tiles[m] = sbtp.tile([P, S], F32R, tag="sbt",
                                                     name=f"sbt{it}_{m}")
                        g_av = [psav.tile([P, 512], F32, tag="av", name=f"pa{it}_{qr}")
                                for qr in range(4)]
                        g_sc = [psc.tile([P, 1024], F32, tag="sc", name=f"pb{it}_{nh}")
                                for nh in range(2)]
                        for k in range(KD):
                            for qr in range(4):
                                nc.tensor.matmul(
                                    g_av[qr][:], w_tiles[0][:, k, :],
                                    xt_sb[k][:, qr * 512:(qr + 1) * 512],
                                    start=(k == 0), stop=(k == KD - 1))
                            for nh in range(2):
                                nc.tensor.matmul(
                                    g_sc[nh][:, 0:512], w_tiles[4][:, k, :],
                                    xt_sb[k][:, nh * 1024: nh * 1024 + 512],
                                    start=(k == 0), stop=(k == KD - 1))
                                nc.tensor.matmul(
                                    g_sc[nh][:, 512:1024], w_tiles[4][:, k, :],
                                    xt_sb[k][:, nh * 1024 + 512:(nh + 1) * 1024],
                                    start=(k == 0), stop=(k == KD - 1))
                        for qr in range(4):
                            nc.vector.tensor_scalar_add(
                                sbt_tiles[0][:, qr * 512:(qr + 1) * 512], g_av[qr][:],
                                bqk_sb[:, 0:1])
                        for nh in range(2):
                            nc.vector.tensor_scalar_add(
                                sbt_tiles[4][:, nh * 1024:(nh + 1) * 1024], g_sc[nh][:],
                                bqk_sb[:, 4:5])

                    def v_quantum(ms, k0=0, nk=KD, first=True, it=it, v_sb=v_sb,
                                  steady=False):
                        """Accumulate nk k-chunks of the V projection for sequence
                        chunk ms into the V' tile (all 8 heads, N=512). Steady
                        (inside-attention) quanta must use the sc slots — the av
                        slots are held by the head's accumulators."""
                        if steady:
                            ps = psc.tile([P, QKC], F32, tag="sc", name=f"pv{it}_{ms}_{k0}")
                        else:
                            ps = psav.tile([P, QKC], F32, tag="av", name=f"pv{it}_{ms}_{k0}")
                        for j, k in enumerate(range(k0, k0 + nk)):
                            nc.tensor.matmul(
                                ps[:], xt_sb[k][:, ms * P:(ms + 1) * P], wv_sb[k][:],
                                start=(j == 0), stop=(j == nk - 1))
                        dst = v_sb[ms][:, :, 0:HD]
                        src3 = ps[:].rearrange("p (h e) -> p h e", e=HD)
                        if first:
                            nc.vector.tensor_tensor(
                                out=dst, in0=src3,
                                in1=bv_bc[:, :].rearrange("p (h e) -> p h e", e=HD),
                                op=ALU.add)
                            nc.vector.tensor_copy(
                                v_sb[ms][:, :, HD:HD + 1],
                                ones8[:, :].rearrange("p (h o) -> p h o", o=1))
                        else:
                            nc.vector.tensor_tensor(out=dst, in0=src3, in1=dst, op=ALU.add)

                    # ---- static quantum schedule ----
                    # sched[(h, kc)] -> quanta emitted inside that chunk, filling
                    # the PE bubble while ScalarE runs the chunk's exps. Only the
                    # remaining m-tile pairs are spread (thinly, ~0.4us/chunk) so
                    # the PE keeps slack to hide PSUM slot handoffs.
                    sched = {}

                    def add(h, kc, fn):
                        sched.setdefault((h, kc), []).append(fn)

                    # V chunks 6-15 just-in-time inside head 0: AV(ms) runs at
                    # chunk ms+1, so producing V(ms) at chunk ms-6 leaves margin
                    for ms in range(2, MS):
                        add(0, ms - 2, lambda ms=ms: v_quantum(ms, k0=0, nk=KD, first=True, steady=True))
                    # m-tile pair 1 during head 1 (head 0 is full with V);
                    # pairs 2/3 during heads 2p-2 / 2p-1 at every 4th chunk
                    for p in (1, 2, 3):
                        quanta = []
                        for quarter in range(4):
                            for m in (p, 4 + p):
                                quanta.append(lambda m=m, quarter=quarter: qk_quantum_fd(m, quarter))
                        for i, fn in enumerate(quanta):
                            if p == 1:
                                add(1, 2 * i, fn)
                            else:
                                add(2 * p - 2 + i // 4, (4 * i + 2) % 16, fn)

                    # prefix (uses the av-pool PSUM slots, which attention has
                    # not claimed yet): m-tiles 0, 4 trickling behind the xt
                    # load, then the V projection single-pass, then pair 1
                    prefix_m0_m4()
                    for ms in range(2):
                        v_quantum(ms, k0=0, nk=KD, first=True)

                    # ---------------- attention ----------------
                    ot_cell = [None]

                    def attention_head(h, it=it, v_sb=v_sb):
                        g = h // 2
                        off = (h % 2) * HD
                        qt = sbt_tiles[g]
                        kt = sbt_tiles[4 + g]

                        avs = [
                            psav.tile([HD + 1, 512], F32, tag="av", name=f"av{it}_{h}_{q}")
                            for q in range(NQ)
                        ]
                        def emit_av(kc, st):
                            for q in range(NQ):
                                nc.tensor.matmul(
                                    avs[q][:], v_sb[kc][:, h, :], st[:, q * 512:(q + 1) * 512],
                                    start=(kc == 0), stop=(kc == MS - 1))

                        # software pipeline: chunk kc emits QK/exp for kc but the
                        # AV matmuls for kc-1, so the in-order PE stream never
                        # waits on ScalarE finishing the current chunk's exp.
                        prev = None
                        for kc in range(MS):
                            st = stp.tile([P, S], F32R, tag="st", name=f"st{it}_{h}_{kc}")
                            for qh in range(2):
                                sc = psc.tile([P, 1024], F32, tag="sc",
                                              name=f"sc{it}_{h}_{kc}_{qh}")
                                nc.tensor.matmul(
                                    sc[:, 0:512],
                                    kt[off:off + HD, kc * P:(kc + 1) * P],
                                    qt[off:off + HD, qh * 1024: qh * 1024 + 512],
                                    start=True, stop=True)
                                nc.tensor.matmul(
                                    sc[:, 512:1024],
                                    kt[off:off + HD, kc * P:(kc + 1) * P],
                                    qt[off:off + HD, qh * 1024 + 512:(qh + 1) * 1024],
                                    start=True, stop=True)
                                nc.scalar.activation(
                                    st[:, qh * 1024:(qh + 1) * 1024], sc[:],
                                    AF.Exp, scale=SCALE)
                            if prev is not None:
                                emit_av(*prev)
                            for fn in sched.pop((h, kc), ()):
                                fn()
                            prev = (kc, st)
                        emit_av(*prev)

                        # normalize: reciprocal of the sums row, broadcast across
                        # partitions into bc, multiply in-place, DMA out per head
                        bc = stp.tile([HD, S], F32, tag="st", name=f"bc{it}_{h}")
                        for q in range(NQ):
                            nc.vector.reciprocal(
                                bc[0:1, q * 512:(q + 1) * 512], avs[q][HD:HD + 1, :])
                            nc.gpsimd.partition_broadcast(
                                bc[:, q * 512:(q + 1) * 512],
                                bc[0:1, q * 512:(q + 1) * 512])
                            nc.vector.tensor_mul(
                                bc[:, q * 512:(q + 1) * 512],
                                avs[q][0:HD, :], bc[:, q * 512:(q + 1) * 512])
                            nc.sync.dma_start(
                                out=outT[h * HD:(h + 1) * HD, q * 512:(q + 1) * 512],
                                in_=bc[:, q * 512:(q + 1) * 512])

                    for h in range(NHC):
                        attention_head(h)
                    assert not sched, f"unemitted quanta: {list(sched)}"

    nc.finalize()
    return nc


_NC_CACHE = {}


def _get_nc(iters=1):
    if iters not in _NC_CACHE:
        _NC_CACHE[iters] = _build(iters)
    return _NC_CACHE[iters]


def _permute_wqk(wqk):
    # [k*128+p, m*128+j] -> [m*128+p, k*128+j]: one m-tile contiguous per row
    w4 = wqk.reshape(KD, P, KD, P)
    return np.ascontiguousarray(w4.transpose(2, 1, 0, 3).reshape(D, D))


def make_in_maps(inputs, W_qkv, b_qkv):
    inputs = np.asarray(inputs, dtype=np.float32)
    W = np.asarray(W_qkv, dtype=np.float32)
    b = np.asarray(b_qkv, dtype=np.float32)
    xt_by_b = [np.ascontiguousarray(inputs[bi].T) for bi in range(B_FULL)]
    in_maps = []
    for c in range(N_CORES):
        bi, hg = c // 2, c % 2
        c0 = hg * QKC
        in_maps.append({
            "xt": xt_by_b[bi],
            "wqk": _permute_wqk(
                np.concatenate([W[:, c0:c0 + QKC], W[:, D + c0: D + c0 + QKC]], axis=1)),
            "wv": np.ascontiguousarray(W[:, 2 * D + c0: 2 * D + c0 + QKC]),
            "bqk": np.ascontiguousarray(
                np.concatenate([b[c0:c0 + QKC], b[D + c0: D + c0 + QKC]])),
            "bv": np.ascontiguousarray(b[2 * D + c0: 2 * D + c0 + QKC]),
        })
    return in_maps


def assemble(results, B=B_FULL):
    out = np.empty((B, S, D), dtype=np.float32)
    for c in range(N_CORES):
        bi, hg = c // 2, c % 2
        out[bi, :, hg * QKC:(hg + 1) * QKC] = np.asarray(results[c]["outT"]).T
    return out


def kernel(inputs, mask, W_qkv, b_qkv):
    # mask is all-True for this problem (spec: fill=ones); it does not affect softmax.
    nc = _get_nc()
    in_maps = make_in_maps(inputs, W_qkv, b_qkv)
    res = run_bass_kernel_spmd(nc, in_maps, core_ids=list(range(N_CORES)))
    return assemble(res.results)



# revision 5
# speedup vs baseline: 1.1844x; 1.1844x over previous
"""Multi-head attention (B=4, S=2048, D=1024, H=16, HD=64) on 8 TRN2 NeuronCores.

Sharding: core c handles batch b = c//2 and head-group hg = c%2 (8 heads each).
Embarrassingly parallel over (b, head-group); QKV projection column-sharded.

v2 dataflow (all on-chip data bf16; PSUM f32):
  - Host passes X^T [D,S], W m-tiles, all bf16 (halves DMA + SBUF vs f32).
  - Projection: Q^T/K^T m-tiles [128, S] = W^T X via full-depth PSUM groups,
    DVE bias-add moves PSUM -> sbt (bf16). V per head: [128 seq, 64] chunks
    + a ones column -> V' [128, 8, 65].
  - Attention in 16 passes p = (head h, q-half qh). Per kpos-chunk kc:
      S^T[kc, qhalf] = K^T.T @ Q^T   (PE, 2x512 free)
      st = exp(S^T/8)                 (ScalarE activation, or GPSIMD pow with
                                       base e^(1/8) via a DVE PSUM->SBUF copy;
                                       split keeps ScalarE off the critical path)
  - AV runs one pass behind (st fully available): out[q, d] orientation —
    stationary st [128,128] slices, moving V' [128, 65]: 65-row streams cost
    half of the q-moving orientation on the PE (cost = moving rows only).
    Per q-chunk: 16-matmul PSUM group in one bank; softmax sums ride the
    ones column; DVE reciprocal + per-partition tensor_scalar_mul normalize
    (no partition broadcast needed).
  - Output staged bf16 [128, 16, 64] per head, one DMA per head; host
    transposes/concatenates and upcasts to f32.

The projection is woven into the pass schedule so the PE never idles:
m-tile pairs (g, 4+g) land before head-group g's first pass; V'(h) lands
before head h's first AV pass. exp chunks alternate ScalarE/GPSIMD.
"""

import numpy as np

import concourse.bass as bass
import concourse.mybir as mybir
import concourse.tile as tile
from concourse import bacc
from concourse.bass_utils import run_bass_kernel_spmd

F32 = mybir.dt.float32
BF16 = mybir.dt.bfloat16
AF = mybir.ActivationFunctionType
ALU = mybir.AluOpType

P = 128          # partitions
D = 1024         # model dim
S = 2048         # sequence
HD = 64          # head dim
NHC = 8          # heads per core
QKC = NHC * HD   # 512 cols per core for each of Q, K, V
KD = D // P      # 8 contraction chunks
MS = S // P      # 16 kpos chunks
QH = 1024        # q-half width
NPASS = 16       # (head, q-half) passes
SCALE = 1.0 / 8.0
EXP_BASE = float(np.exp(SCALE))

N_CORES = 8
B_FULL = 4

# kc steps whose exp chunk runs on GPSIMD (pow) instead of ScalarE
POOL_KC = (1, 4, 7, 10, 13)


def _build(iters=1):
    nc = bacc.Bacc(None, target_bir_lowering=False)

    xt = nc.dram_tensor("xt", [D, S], BF16, kind="ExternalInput")
    # wqk host-permuted: row (m*128 + p), col (k*128 + j) holds
    # W_qk[k*128 + p, m*128 + j] — one m-tile = contiguous [128, 1024] block
    wqk = nc.dram_tensor("wqk", [D, D], BF16, kind="ExternalInput")
    wv = nc.dram_tensor("wv", [D, QKC], BF16, kind="ExternalInput")
    bqk = nc.dram_tensor("bqk", [P, KD], F32, kind="ExternalInput")   # [p, m]
    bvb = nc.dram_tensor("bvb", [P, QKC], F32, kind="ExternalInput")  # replicated
    outd = nc.dram_tensor("outd", [NHC * P, MS * HD], BF16, kind="ExternalOutput")

    with tile.TileContext(nc) as tc:
        with (
            tc.tile_pool(name="persist", bufs=1) as pp,
            tc.tile_pool(name="psc", bufs=2, space="PSUM") as psc,
            tc.tile_pool(name="psav", bufs=2, space="PSUM") as psav,
            tc.tile_pool(name="pspj", bufs=2, space="PSUM") as pspj,
        ):
            for it in range(iters):
                bqk_sb = pp.tile([P, KD], F32, tag="bqk", name=f"bqk{it}")
                bvb_sb = pp.tile([P, QKC], F32, tag="bvb", name=f"bvb{it}")
                expbase = pp.tile([P, QH], F32, tag="eb", name=f"eb{it}")
                nc.sync.dma_start(out=bqk_sb[:], in_=bqk[:])
                nc.scalar.dma_start(out=bvb_sb[:], in_=bvb[:])
                nc.vector.memset(expbase[:], EXP_BASE)

                # --- input DMA on two HWDGE queues (sync + scalar) ---
                xt_sb = [pp.tile([P, S], BF16, tag=f"xt{k}", name=f"xt{it}_{k}")
                         for k in range(KD)]
                w_sb = [pp.tile([P, KD, P], BF16, tag=f"wm{m}", name=f"wm{it}_{m}")
                        for m in range(KD)]
                wv_sb = [pp.tile([P, QKC], BF16, tag=f"wv{k}", name=f"wv{it}_{k}")
                        for k in range(KD)]

                def dma_w(m, eng, it=it):
                    eng.dma_start(
                        out=w_sb[m][:],
                        in_=wqk[m * P:(m + 1) * P, :].rearrange(
                            "p (k j) -> p k j", k=KD))

                # sync queue: even xt; scalar queue: w0/w4 then odd xt,
                # then wv, then remaining w m-tiles
                for k in range(0, KD, 2):
                    nc.sync.dma_start(out=xt_sb[k][:], in_=xt[k * P:(k + 1) * P, :])
                dma_w(0, nc.scalar)
                dma_w(4, nc.scalar)
                for k in range(1, KD, 2):
                    nc.scalar.dma_start(out=xt_sb[k][:], in_=xt[k * P:(k + 1) * P, :])
                for k in range(KD):
                    (nc.sync if k % 2 == 0 else nc.scalar).dma_start(
                        out=wv_sb[k][:], in_=wv[k * P:(k + 1) * P, :])
                for m in (1, 5, 2, 6, 3, 7):
                    dma_w(m, nc.sync if m % 2 == 0 else nc.scalar)

                # persistent attention tensors
                sbt = [pp.tile([P, S], BF16, tag=f"sbt{m}", name=f"sbt{it}_{m}")
                       for m in range(KD)]
                vv = [pp.tile([P, NHC, HD + 1], BF16, tag=f"vv{k}",
                              name=f"vv{it}_{k}") for k in range(MS)]
                out_sb = [pp.tile([P, MS * HD], BF16, tag=f"ou{h}",
                                  name=f"ou{it}_{h}") for h in range(NHC)]

                # ---------- projection quanta ----------
                def m_quantum(m, qr, slot="pj", it=it):
                    """Full-depth [128,512] quarter of Q/K m-tile m."""
                    if slot == "pj":
                        ps = pspj.tile([P, 512], F32, tag="pj",
                                       name=f"pm{it}_{m}_{qr}")
                        dst = ps[:]
                    else:  # use lower bank of an sc tile
                        ps = psc.tile([P, QH], F32, tag="sc",
                                      name=f"pm{it}_{m}_{qr}")
                        dst = ps[:, 0:512]
                    for k in range(KD):
                        nc.tensor.matmul(dst, w_sb[m][:, k, :],
                                         xt_sb[k][:, qr * 512:(qr + 1) * 512],
                                         start=(k == 0), stop=(k == KD - 1))
                    nc.vector.tensor_scalar_add(
                        sbt[m][:, qr * 512:(qr + 1) * 512], dst,
                        bqk_sb[:, m:m + 1])

                def m_quantum_part(m, qr, k0, scr, it=it):
                    """Half-depth quarter (prologue): k0..k0+3 into PSUM, then
                    DVE add with bias (k0=0, into f32 scratch) or with the
                    scratch (k0=4, into sbt bf16)."""
                    ps = pspj.tile([P, 512], F32, tag="pj",
                                   name=f"pp{it}_{m}_{qr}_{k0}")
                    for j, k in enumerate(range(k0, k0 + 4)):
                        nc.tensor.matmul(ps[:], w_sb[m][:, k, :],
                                         xt_sb[k][:, qr * 512:(qr + 1) * 512],
                                         start=(j == 0), stop=(j == 3))
                    if k0 == 0:
                        nc.vector.tensor_scalar_add(scr[:], ps[:],
                                                    bqk_sb[:, m:m + 1])
                    else:
                        nc.vector.tensor_tensor(
                            out=sbt[m][:, qr * 512:(qr + 1) * 512],
                            in0=ps[:], in1=scr[:], op=ALU.add)

                def v_quantum(h, kc, slot="pj", it=it):
                    """V' chunk kc for head h: [128 seq, 64] + bias."""
                    if slot == "pj":
                        ps = pspj.tile([P, 512], F32, tag="pj",
                                       name=f"pv{it}_{h}_{kc}")
                        dst = ps[:, 0:HD]
                    else:
                        ps = psav.tile([P, HD + 1], F32, tag="av",
                                       name=f"pv{it}_{h}_{kc}")
                        dst = ps[:, 0:HD]
                    for k in range(KD):
                        nc.tensor.matmul(dst, xt_sb[k][:, kc * P:(kc + 1) * P],
                                         wv_sb[k][:, h * HD:(h + 1) * HD],
                                         start=(k == 0), stop=(k == KD - 1))
                    nc.vector.tensor_tensor(
                        out=vv[kc][:, h, 0:HD], in0=dst,
                        in1=bvb_sb[:, h * HD:(h + 1) * HD], op=ALU.add)
                    if h == 0:
                        nc.vector.memset(vv[kc][:, :, HD:HD + 1], 1.0)

                # ---------- static weave schedule ----------
                sched = {}

                def add(p, kc, fn):
                    sched.setdefault((p, kc), []).append(fn)

                # m-tile pairs (g, 4+g): 2 quanta/pass over passes 4(g-1)..+3
                for g in (1, 2, 3):
                    quanta = [(m, qr) for qr in range(4) for m in (g, 4 + g)]
                    for i, (m, qr) in enumerate(quanta):
                        add(4 * (g - 1) + i // 2, 5 + 8 * (i % 2),
                            lambda m=m, qr=qr: m_quantum(m, qr, slot="pj"))
                # V'(h): h=1 in pass 0 (one chunk per kc); h>=2 split over
                # passes 2h-3 / 2h-2 (8 chunks each, odd kc)
                for kc in range(MS):
                    add(0, kc, lambda kc=kc: v_quantum(1, kc, slot="pj"))
                for h in range(2, NHC):
                    for i in range(MS):
                        add(2 * h - 3 + i // 8, 2 * (i % 8) + 1,
                            lambda h=h, kc=i: v_quantum(h, kc, slot="pj"))

                # ---------- prologue ----------
                with tc.tile_pool(name=f"prolog{it}", bufs=1) as plp:
                    scrm = {}
                    for m in (0, 4):
                        for qr in range(4):
                            scrm[(m, qr)] = plp.tile(
                                [P, 512], F32, tag=f"scrm{m}_{qr}",
                                name=f"scrm{it}_{m}_{qr}")
                    for k0 in (0, 4):
                        for m in (0, 4):
                            for qr in range(4):
                                m_quantum_part(m, qr, k0, scrm[(m, qr)])
                    for kc in range(MS):
                        v_quantum(0, kc, slot="av" if kc % 2 else "pj")

                # ---------- attention passes ----------
                stp = tc.alloc_tile_pool(name=f"stp{it}", bufs=1)
                st_tiles = [[None] * MS, [None] * MS]
                av_cur = [None] * (QH // P)
                recp = stp

                def emit_qk(p, kc, it=it):
                    h, qh = p // 2, p % 2
                    g, off = h // 2, (h % 2) * HD
                    qt, kt = sbt[g], sbt[4 + g]
                    sc = psc.tile([P, QH], F32, tag="sc", name=f"sc{it}_{p}_{kc}")
                    for j in range(2):
                        nc.tensor.matmul(
                            sc[:, j * 512:(j + 1) * 512],
                            kt[off:off + HD, kc * P:(kc + 1) * P],
                            qt[off:off + HD, qh * QH + j * 512: qh * QH + (j + 1) * 512],
                            start=True, stop=True)
                    st = stp.tile([P, QH], BF16, tag="st", bufs=32,
                                  name=f"st{it}_{p}_{kc}")
                    st_tiles[p % 2][kc] = st
                    if kc in POOL_KC:
                        scr = stp.tile([P, QH], F32, tag="scr", bufs=3,
                                       name=f"scr{it}_{p}_{kc}")
                        nc.vector.tensor_copy(scr[:], sc[:])
                        nc.gpsimd.tensor_tensor(out=st[:], in0=expbase[:],
                                                in1=scr[:], op=ALU.pow)
                    else:
                        nc.scalar.activation(st[:], sc[:], AF.Exp, scale=SCALE)

                def emit_av(pprev, kc, it=it):
                    h, qh = pprev // 2, pprev % 2
                    qc, khalf = kc // 2, kc % 2
                    if khalf == 0:
                        av_cur[qc] = psav.tile([P, HD + 1], F32, tag="av",
                                               name=f"av{it}_{pprev}_{qc}")
                    ps = av_cur[qc]
                    stt = st_tiles[pprev % 2]
                    for k2 in range(8):
                        kcc = khalf * 8 + k2
                        nc.tensor.matmul(ps[:], stt[kcc][:, qc * P:(qc + 1) * P],
                                         vv[kcc][:, h, :],
                                         start=(kcc == 0), stop=(kcc == MS - 1))
                    if khalf == 1:
                        qg = qh * 8 + qc
                        rec = recp.tile([P, 1], F32, tag="rec", bufs=4,
                                        name=f"rec{it}_{pprev}_{qc}")
                        nc.vector.reciprocal(rec[:], ps[:, HD:HD + 1])
                        nc.vector.tensor_scalar_mul(
                            out_sb[h][:, qg * HD:(qg + 1) * HD],
                            ps[:, 0:HD], rec[:])

                for p in range(NPASS + 1):
                    for kc in range(MS):
                        if p < NPASS:
                            emit_qk(p, kc)
                        if p >= 1:
                            emit_av(p - 1, kc)
                        for fn in sched.pop((p, kc), ()):
                            fn()
                    if p >= 2 and p % 2 == 0:
                        h = (p - 2) // 2
                        nc.sync.dma_start(out=outd[h * P:(h + 1) * P, :],
                                          in_=out_sb[h][:])
                assert not sched, f"unemitted quanta: {list(sched)}"
                stp.release()

    nc.finalize()
    return nc


_NC_CACHE = {}


def _get_nc(iters=1):
    if iters not in _NC_CACHE:
        _NC_CACHE[iters] = _build(iters)
    return _NC_CACHE[iters]


def _permute_wqk(w):
    # [k*128+p, m*128+j] -> [m*128+p, k*128+j]: one m-tile contiguous per row
    w4 = w.reshape(KD, P, KD, P)
    return np.ascontiguousarray(w4.transpose(2, 1, 0, 3).reshape(D, D))


def make_in_maps(inputs, W_qkv, b_qkv):
    import ml_dtypes
    BF = ml_dtypes.bfloat16
    inputs = np.asarray(inputs, dtype=np.float32)
    W = np.asarray(W_qkv, dtype=np.float32)
    b = np.asarray(b_qkv, dtype=np.float32)
    xt_by_b = [np.ascontiguousarray(inputs[bi].T).astype(BF)
               for bi in range(B_FULL)]
    in_maps = []
    for c in range(N_CORES):
        bi, hg = c // 2, c % 2
        c0 = hg * QKC
        bqk_cat = np.concatenate([b[c0:c0 + QKC], b[D + c0:D + c0 + QKC]])
        in_maps.append({
            "xt": xt_by_b[bi],
            "wqk": _permute_wqk(np.concatenate(
                [W[:, c0:c0 + QKC], W[:, D + c0:D + c0 + QKC]],
                axis=1)).astype(BF),
            "wv": np.ascontiguousarray(W[:, 2 * D + c0:2 * D + c0 + QKC]).astype(BF),
            "bqk": np.ascontiguousarray(bqk_cat.reshape(KD, P).T),
            "bvb": np.ascontiguousarray(np.broadcast_to(
                b[2 * D + c0:2 * D + c0 + QKC], (P, QKC))),
        })
    return in_maps


def assemble(results, B=B_FULL):
    out = np.empty((B, S, D), dtype=np.float32)
    for c in range(N_CORES):
        bi, hg = c // 2, c % 2
        arr = np.asarray(results[c]["outd"]).reshape(NHC, P, MS, HD)
        arr = arr.transpose(2, 1, 0, 3).reshape(S, QKC)
        out[bi, :, hg * QKC:(hg + 1) * QKC] = arr.astype(np.float32)
    return out


def kernel(inputs, mask, W_qkv, b_qkv):
    # mask is all-True for this problem (spec: fill=ones); softmax unaffected.
    nc = _get_nc()
    in_maps = make_in_maps(inputs, W_qkv, b_qkv)
    res = run_bass_kernel_spmd(nc, in_maps, core_ids=list(range(N_CORES)))
    return assemble(res.results)
